# revision 1
# baseline (speedup 1.0000x reference)
import numpy as np

import bass_rust
import concourse.bass as bass
import concourse.tile as tile
import concourse.mybir as mybir
from concourse.bass_utils import run_bass_kernel_spmd

B, S, D = 2, 2048, 2048
NH, NKV, HD = 16, 4, 128
GQ = 512
NT = S // 128
NKO = D // 128
PC = 256
NPC = S // PC
QC = 512
NQC = S // QC
MAGIC = float(np.float32(12582912.0))
SM_SCALE = 1.0 / float(np.sqrt(HD))

F32 = mybir.dt.float32
F32R = mybir.dt.float32r
MULT = mybir.AluOpType.mult
ADD = mybir.AluOpType.add
DIV = mybir.AluOpType.divide
EXP = mybir.ActivationFunctionType.Exp

_CACHE = {}

LAST_RESULTS = None


def _split_multi_waits(nc):
    for f in nc.m.functions:
        for bb in f.blocks:
            new = []
            for inst in bb.instructions:
                si = inst.sync_info
                if si is None:
                    new.append(inst)
                    continue
                waits = list(si.on_wait)
                if len(waits) > 1:
                    for k, w in enumerate(waits[:-1]):
                        nop = mybir.InstNoOp(name=f"{inst.name}-w{k}", ins=[], outs=[])
                        nop.engine = inst.engine
                        nop.sync_info = bass_rust.SyncInfo(on_wait=[w], on_update=[])
                        new.append(nop)
                    inst.sync_info = bass_rust.SyncInfo(
                        on_wait=[waits[-1]], on_update=list(si.on_update)
                    )
                new.append(inst)
            bb.instructions = new


def _host_consts():
    theta = 10000.0
    angles = 1.0 / theta ** (np.arange(0, HD, 2, dtype=np.float32) / HD)
    emb = np.outer(np.arange(S, dtype=np.float32), angles)
    emb = np.concatenate([emb, emb], axis=-1)
    cosT = np.ascontiguousarray(np.cos(emb).T).astype(np.float32)
    sinT = np.ascontiguousarray(np.sin(emb).T).astype(np.float32)

    rot = np.zeros((128, 128), dtype=np.float32)
    for i in range(64):
        rot[i, i + 64] = 1.0
        rot[i + 64, i] = -1.0

    masks = np.zeros((128, 4, QC), dtype=np.float32)
    p = np.arange(128)[:, None]
    fidx = np.arange(QC)[None, :]
    for j in range(4):
        masks[:, j, :] = (128 * j + p <= fidx).astype(np.float32)
    masks = np.ascontiguousarray(masks.reshape(128, 4 * QC))

    ones = np.ones((128, 128), dtype=np.float32)
    ident = np.eye(128, dtype=np.float32)
    return {
        "cosT": cosT, "sinT": sinT, "rot": rot,
        "masks": masks, "ones": ones, "ident": ident,
    }



def _act_reciprocal(nc, out, in_):
    eng = nc.scalar
    inputs = [eng.lower_ap(in_)]
    for val in (0.0, 1.0, 0.0):
        inputs.append(mybir.ImmediateValue(dtype=mybir.dt.float32, value=val))
    return eng.add_instruction(
        mybir.InstActivation(
            name=nc.get_next_instruction_name(),
            func=mybir.ActivationFunctionType.Reciprocal,
            ins=inputs,
            outs=[eng.lower_ap(out)],
        )
    )

def _build_nc():
    nc = bass.Bass("TRN2", target_bir_lowering=False, debug=False)

    dataT = nc.dram_tensor("dataT", [D, S], F32R, kind="ExternalInput").ap()
    wq = nc.dram_tensor("wq", [D, GQ], F32R, kind="ExternalInput").ap()
    wk = nc.dram_tensor("wk", [D, HD], F32R, kind="ExternalInput").ap()
    wv = nc.dram_tensor("wv", [D, HD], F32R, kind="ExternalInput").ap()
    wo = nc.dram_tensor("wo", [GQ, D], F32R, kind="ExternalInput").ap()
    cosT_d = nc.dram_tensor("cosT", [128, S], F32, kind="ExternalInput").ap()
    sinT_d = nc.dram_tensor("sinT", [128, S], F32, kind="ExternalInput").ap()
    rot_d = nc.dram_tensor("rot", [128, 128], F32R, kind="ExternalInput").ap()
    masks_d = nc.dram_tensor("masks", [128, 4 * QC], F32R, kind="ExternalInput").ap()
    ones_d = nc.dram_tensor("ones", [128, 128], F32R, kind="ExternalInput").ap()
    ident_d = nc.dram_tensor("ident", [128, 128], F32R, kind="ExternalInput").ap()
    outT = nc.dram_tensor("outT", [D, S], F32, kind="ExternalOutput").ap()

    dataT_r = dataT.rearrange("(ko p) t -> p ko t", p=128)
    wq_r = wq.rearrange("(ko p) m -> p ko m", p=128)
    wk_r = wk.rearrange("(ko p) m -> p ko m", p=128)
    wv_r = wv.rearrange("(ko p) m -> p ko m", p=128)
    wo_r = wo.rearrange("(h p) n -> p h n", p=128)

    from contextlib import ExitStack
    with tile.TileContext(nc) as tc, ExitStack() as stack:
        consts = stack.enter_context(tc.tile_pool(name="consts", bufs=1))
        cos_sb = consts.tile([128, S], F32)
        sin_sb = consts.tile([128, S], F32)
        rot_sb = consts.tile([128, 128], F32R)
        ones_sb = consts.tile([128, 128], F32R)
        id_sb = consts.tile([128, 128], F32R)

        persist = stack.enter_context(tc.tile_pool(name="persist", bufs=1))
        xq_h = [persist.tile([128, S], F32R, tag=f"xqh{h}", name=f"xq_h{h}")
                for h in range(4)]
        kT_g = [persist.tile([128, 512], F32R, tag=f"ktg{g}", name=f"kT_g{g}")
                for g in range(4)]
        v_g = [persist.tile([128, 4, HD], F32R, tag=f"vg{g}", name=f"v_g{g}")
               for g in range(4)]

        kvpool = stack.enter_context(tc.tile_pool(name="kvpool", bufs=1))
        xkT_roped = kvpool.tile([128, S], F32R)
        xvT_raw = kvpool.tile([128, S], F32R)

        with tc.tile_pool(name="rawpool", bufs=1) as rawpool:
            xqT_raw = [rawpool.tile([128, S], F32R, tag=f"xq{h}", name=f"xqT_raw{h}")
                       for h in range(4)]
            xkT_raw = rawpool.tile([128, S], F32R)

            with tc.tile_pool(name="wpool", bufs=1) as wpool, \
                 tc.tile_pool(name="datapool", bufs=2) as datapool, \
                 tc.tile_pool(name="t2pool", bufs=2) as t2pool, \
                 tc.tile_pool(name="proj_ps", bufs=3, space="PSUM") as proj_ps, \
                 tc.tile_pool(name="rope_ps", bufs=2, space="PSUM") as rope_ps:
                wq_t = [wpool.tile([128, 4, GQ], F32R, tag=f"wq{g}", name=f"wq{g}")
                        for g in range(4)]
                wk_sb = wpool.tile([128, NKO, HD], F32R)
                wv_sb = wpool.tile([128, NKO, HD], F32R)
                dT = {}
                t_ = datapool.tile([128, NKO, PC], F32R, tag="dT", name="dT0")
                dT[0] = t_
                t1_ = datapool.tile([128, NKO, PC], F32R, tag="dT", name="dT1")
                dT[1] = t1_
                nc.sync.dma_start(wq_t[0][:], wq_r[:, 0:4])
                nc.sync.dma_start(t_[:, 0:8], dataT_r[:, 0:8, bass.ts(0, PC)])
                nc.sync.dma_start(wq_t[1][:], wq_r[:, 4:8])
                nc.sync.dma_start(t_[:, 8:16], dataT_r[:, 8:16, bass.ts(0, PC)])
                nc.sync.dma_start(wq_t[2][:], wq_r[:, 8:12])
                nc.sync.dma_start(t1_[:, 0:8], dataT_r[:, 0:8, bass.ts(1, PC)])
                nc.sync.dma_start(wq_t[3][:], wq_r[:, 12:16])
                nc.sync.dma_start(t1_[:, 8:16], dataT_r[:, 8:16, bass.ts(1, PC)])
                nc.sync.dma_start(wk_sb[:], wk_r[:])
                nc.sync.dma_start(wv_sb[:], wv_r[:])
                nc.sync.dma_start(cos_sb[:], cosT_d[:])
                nc.sync.dma_start(sin_sb[:], sinT_d[:])
                nc.sync.dma_start(rot_sb[:], rot_d[:])
                nc.sync.dma_start(ones_sb[:], ones_d[:])
                nc.sync.dma_start(id_sb[:], ident_d[:])

                def rope_chunk(src_ap, dst_ap, ccsl):
                    pr = rope_ps.tile([128, QC], F32, tag="rps")
                    nc.tensor.matmul(pr[:], rot_sb[:], src_ap, start=True, stop=True)
                    t1 = rope_ps.tile([128, QC], F32, tag="t1")
                    t2 = t2pool.tile([128, QC], F32, tag="t2")
                    nc.vector.tensor_tensor(t1[:], src_ap, cos_sb[:, ccsl], MULT)
                    nc.vector.tensor_tensor(t2[:], pr[:], sin_sb[:, ccsl], MULT)
                    nc.vector.tensor_tensor(dst_ap, t1[:], t2[:], ADD)

                def _lhsT(kind, h, ko):
                    if kind == "q":
                        return wq_t[ko // 4][:, ko % 4, bass.ts(h, 128)]
                    if kind == "k":
                        return wk_sb[:, ko]
                    return wv_sb[:, ko]

                def _copy_out(kind, h, csl, ps):
                    if kind == "q":
                        nc.vector.tensor_copy(xqT_raw[h][:, csl], ps[:])
                    elif kind == "k":
                        nc.vector.tensor_copy(xkT_raw[:, csl], ps[:])
                    else:
                        nc.vector.tensor_copy(xvT_raw[:, csl], ps[:])

                QTGT = [("q", h) for h in range(4)]
                KVTGT = [("k", 0), ("v", 0)]

                def emit_proj_chunk(c, targets=None, waves=1, post=None):
                    csl = bass.ts(c, PC)
                    if c + 2 < NPC:
                        cb = c + 2
                        t_ = datapool.tile([128, NKO, PC], F32R, tag="dT",
                                           name=f"dT{cb}")
                        dT[cb] = t_
                        eng = nc.gpsimd if cb % 2 else nc.sync
                        eng.dma_start(t_[:], dataT_r[:, :, bass.ts(cb, PC)])
                    if targets is None:
                        targets = QTGT + KVTGT
                    nw = len(targets) if waves == 1 else waves
                    per = (len(targets) + nw - 1) // nw
                    for w in range(nw):
                        wave = targets[w * per:(w + 1) * per]
                        if not wave:
                            continue
                        pss_w = {t: proj_ps.tile([128, PC], F32, tag="pp",
                                             name=f"pp{c}_{t[0]}{t[1]}")
                                 for t in wave}
                        for ko in range(NKO):
                            for t in wave:
                                nc.tensor.matmul(pss_w[t][:], _lhsT(t[0], t[1], ko),
                                                 dT[c][:, ko],
                                                 start=(ko == 0),
                                                 stop=(ko == NKO - 1))
                        for t in wave:
                            _copy_out(t[0], t[1], csl, pss_w[t])
                            if post is not None:
                                post(t)

                emit_proj_chunk(0, waves=2)
                emit_proj_chunk(1, waves=2)
                for c in range(2, 6):
                    emit_proj_chunk(c)
                for cc in range(3):
                    ccsl = bass.ts(cc, QC)
                    rope_chunk(xkT_raw[:, ccsl], xkT_roped[:, ccsl], ccsl)
                for h in range(4):
                    for cc in range(3):
                        ccsl = bass.ts(cc, QC)
                        rope_chunk(xqT_raw[h][:, ccsl], xq_h[h][:, ccsl], ccsl)
                emit_proj_chunk(6, targets=KVTGT + QTGT)
                cc3 = bass.ts(3, QC)

                def _rope_after(t):
                    kind, h = t
                    if kind == "k":
                        rope_chunk(xkT_raw[:, cc3], xkT_roped[:, cc3], cc3)
                    elif kind == "q":
                        rope_chunk(xqT_raw[h][:, cc3], xq_h[h][:, cc3], cc3)

                emit_proj_chunk(7, targets=KVTGT + QTGT, post=_rope_after)

        with tc.tile_pool(name="p2", bufs=1) as p2, \
             tc.tile_pool(name="qtmp", bufs=2) as qtmp, \
             tc.tile_pool(name="attn_sb", bufs=8) as attn_sb, \
             tc.tile_pool(name="exp_pool", bufs=12) as exp_pool, \
             tc.tile_pool(name="small", bufs=4) as small, \
             tc.tile_pool(name="outstage", bufs=4) as outstage, \
             tc.tile_pool(name="tp_ps", bufs=1, space="PSUM") as tp_ps, \
             tc.tile_pool(name="score_ps", bufs=2, space="PSUM") as score_ps, \
             tc.tile_pool(name="attn_ps", bufs=3, space="PSUM") as attn_ps, \
             tc.tile_pool(name="sum_ps", bufs=2, space="PSUM") as sum_ps:
            out_ps = score_ps
            mask_sb = p2.tile([128, 4 * QC], F32R)
            nc.sync.dma_start(mask_sb[:], masks_d[:])
            wo_t = [p2.tile([128, S], F32R, tag=f"wo{h}", name=f"wo{h}")
                    for h in range(4)]
            for h in range(4):
                nc.sync.dma_start(wo_t[h][:], wo_r[:, h])

            GRP = 4
            def quant_group(src_g, dst_ap):
                amax = qtmp.tile([128, GRP, 1], F32, tag="amax")
                scl = qtmp.tile([128, GRP, 1], F32, tag="scl")
                inv = qtmp.tile([128, GRP, 1], F32, tag="inv")
                xs = qtmp.tile([128, GRP, HD], F32, tag="xs")
                nc.vector.tensor_reduce(amax[:], src_g[:], mybir.AxisListType.X,
                                        mybir.AluOpType.max,
                                        apply_absolute_value=True)
                nc.vector.tensor_scalar_max(amax[:], amax[:], 1e-8)
                nc.vector.tensor_scalar_mul(scl[:], amax[:], 1.0 / 127.0)
                nc.vector.reciprocal(inv[:], scl[:])
                sclb = scl[:].to_broadcast((128, GRP, HD))
                invb = inv[:].to_broadcast((128, GRP, HD))
                nc.vector.tensor_tensor(xs[:], src_g[:], invb, MULT)
                nc.vector.tensor_scalar_add(xs[:], xs[:], MAGIC)
                nc.vector.tensor_scalar_add(xs[:], xs[:], -MAGIC)
                nc.vector.tensor_tensor(dst_ap, xs[:], sclb, MULT)

            def emit_quant_group(grp):
                k_td = qtmp.tile([128, GRP, HD], F32, tag="ktd")
                for j in range(GRP):
                    ti = grp * GRP + j
                    pt = tp_ps.tile([128, 128], F32R, tag="tp")
                    nc.tensor.transpose(pt[:], xkT_roped[:, bass.ts(ti, 128)],
                                        id_sb[:])
                    nc.scalar.copy(k_td[:, j, :], pt[:])
                kq = qtmp.tile([128, GRP, HD], F32R, tag="kq")
                quant_group(k_td, kq[:])
                for j in range(GRP):
                    pt = tp_ps.tile([128, 128], F32R, tag="tp")
                    nc.tensor.transpose(pt[:], kq[:, j, :], id_sb[:])
                    nc.vector.tensor_copy(kT_g[grp][:, bass.ts(j, 128)], pt[:])
                v_td = qtmp.tile([128, GRP, HD], F32, tag="vtd")
                for j in range(GRP):
                    ti = grp * GRP + j
                    pt = tp_ps.tile([128, 128], F32R, tag="tp")
                    nc.tensor.transpose(pt[:], xvT_raw[:, bass.ts(ti, 128)],
                                        id_sb[:])
                    nc.scalar.copy(v_td[:, j, :], pt[:])
                quant_group(v_td, v_g[grp][:])

            def out_proj(c_prev, tiles):
                cpsl = bass.ts(c_prev, QC)
                for dt_ in range(NKO):
                    po = out_ps.tile([128, QC], F32, tag="ps")
                    for h2 in range(4):
                        nc.tensor.matmul(po[:], wo_t[h2][:, bass.ts(dt_, 128)],
                                         tiles[h2][:],
                                         start=(h2 == 0), stop=(h2 == 3))
                    ot = outstage.tile([128, QC], F32, tag="ot")
                    if dt_ % 2 == 0:
                        nc.scalar.copy(ot[:], po[:])
                    else:
                        nc.vector.tensor_copy(ot[:], po[:])
                    nc.sync.dma_start(outT[bass.ts(dt_, 128), cpsl], ot[:])

            prev = None
            for c in range(NQC):
                emit_quant_group(c)
                csl = bass.ts(c, QC)
                nki = 4 * (c + 1)
                attn_tiles = {}
                for h in range(4):
                    pa = attn_ps.tile([128, QC], F32, tag="pa")
                    pss = sum_ps.tile([128, QC], F32, tag="pss")
                    for ki in range(nki):
                        if ki >= 4 * c:
                            j = ki - 4 * c
                            qoff = min(128 * j, 256)
                        else:
                            j, qoff = -1, 0
                        w = QC - qoff
                        ps = score_ps.tile([128, QC], F32, tag="ps")
                        nc.tensor.matmul(ps[:, qoff:], kT_g[ki // 4][:, bass.ts(ki % 4, 128)],
                                         xq_h[h][:, bass.ds(c * QC + qoff, w)],
                                         start=True, stop=True)
                        et = exp_pool.tile([128, QC], F32R, tag="et")
                        nc.scalar.activation(et[:, qoff:], ps[:, qoff:], EXP,
                                             scale=SM_SCALE)
                        if j >= 0:
                            nc.vector.tensor_tensor(
                                et[:, qoff:], et[:, qoff:],
                                mask_sb[:, bass.ds(j * QC + qoff, w)], MULT)
                        nc.tensor.matmul(pss[:, qoff:], ones_sb[:], et[:, qoff:],
                                         start=(ki == 0), stop=(ki == nki - 1))
                        nc.tensor.matmul(pa[:, qoff:], v_g[ki // 4][:, ki % 4],
                                         et[:, qoff:],
                                         start=(ki == 0), stop=(ki == nki - 1))
                    rc = small.tile([128, QC], F32, tag="rc")
                    nc.vector.reciprocal(rc[:], pss[:])
                    at = attn_sb.tile([128, QC], F32R, tag="attnT")
                    attn_tiles[h] = at
                    nc.vector.tensor_tensor(at[:], pa[:], rc[:], MULT)
                    if h == 1 and prev is not None:
                        out_proj(prev[0], prev[1])
                prev = (c, attn_tiles)
            out_proj(prev[0], prev[1])

    _split_multi_waits(nc)
    return nc

def _get_state():
    if "nc" not in _CACHE:
        _CACHE["nc"] = _build_nc()
        _CACHE["consts"] = _host_consts()
    return _CACHE["nc"], _CACHE["consts"]


def kernel(data=None, mask=None, wq=None, wk=None, wv=None, wo=None, **extra):
    global LAST_RESULTS
    nc, consts = _get_state()

    data = np.asarray(data, dtype=np.float32)
    wq = np.asarray(wq, dtype=np.float32)
    wk = np.asarray(wk, dtype=np.float32)
    wv = np.asarray(wv, dtype=np.float32)
    wo = np.asarray(wo, dtype=np.float32)

    in_maps = []
    dTs = [np.ascontiguousarray(data[b].T) for b in range(B)]
    for b in range(B):
        for g in range(NKV):
            in_maps.append({
                "dataT": dTs[b],
                "wq": np.ascontiguousarray(wq[:, g * GQ:(g + 1) * GQ]),
                "wk": np.ascontiguousarray(wk[:, g * HD:(g + 1) * HD]),
                "wv": np.ascontiguousarray(wv[:, g * HD:(g + 1) * HD]),
                "wo": np.ascontiguousarray(wo[g * GQ:(g + 1) * GQ, :]),
                "cosT": consts["cosT"],
                "sinT": consts["sinT"],
                "rot": consts["rot"],
                "masks": consts["masks"],
                "ones": consts["ones"],
                "ident": consts["ident"],
            })

    res = run_bass_kernel_spmd(nc, in_maps, core_ids=list(range(8)))
    LAST_RESULTS = res

    out = np.empty((B, S, D), dtype=np.float32)
    for b in range(B):
        acc = res.results[b * NKV]["outT"].astype(np.float32).copy()
        for g in range(1, NKV):
            acc += res.results[b * NKV + g]["outT"]
        out[b] = acc.T
    return out



# revision 4
# speedup vs baseline: 1.1272x; 1.1272x over previous
import numpy as np

import bass_rust
import concourse.bass as bass
import concourse.tile as tile
import concourse.mybir as mybir
from concourse.bass_utils import run_bass_kernel_spmd

B, S, D = 2, 2048, 2048
NH, NKV, HD = 16, 4, 128
GQ = 512
NKO = D // 128
PC = 512
NPC = S // PC
QC = 512
NQC = S // QC
MAGIC = float(np.float32(12582912.0))
SM_SCALE = 1.0 / float(np.sqrt(HD))

F32 = mybir.dt.float32
F32R = mybir.dt.float32r
BF16 = mybir.dt.bfloat16
MULT = mybir.AluOpType.mult
ADD = mybir.AluOpType.add
EXP = mybir.ActivationFunctionType.Exp

_CACHE = {}

LAST_RESULTS = None


def _split_multi_waits(nc):
    for f in nc.m.functions:
        for bb in f.blocks:
            new = []
            for inst in bb.instructions:
                si = inst.sync_info
                if si is None:
                    new.append(inst)
                    continue
                waits = list(si.on_wait)
                if len(waits) > 1:
                    for k, w in enumerate(waits[:-1]):
                        nop = mybir.InstNoOp(name=f"{inst.name}-w{k}", ins=[], outs=[])
                        nop.engine = inst.engine
                        nop.sync_info = bass_rust.SyncInfo(on_wait=[w], on_update=[])
                        new.append(nop)
                    inst.sync_info = bass_rust.SyncInfo(
                        on_wait=[waits[-1]], on_update=list(si.on_update)
                    )
                new.append(inst)
            bb.instructions = new


def _host_consts():
    theta = 10000.0
    angles = 1.0 / theta ** (np.arange(0, HD, 2, dtype=np.float32) / HD)
    emb = np.outer(np.arange(S, dtype=np.float32), angles)
    emb = np.concatenate([emb, emb], axis=-1)
    cos = np.cos(emb).astype(np.float32)
    sin = np.sin(emb).astype(np.float32)
    cosT = np.ascontiguousarray(cos.T)
    sinT = np.ascontiguousarray(sin.T)

    ctd = np.ascontiguousarray(cos.reshape(S // 128, 128, HD).transpose(1, 0, 2))
    std = sin.reshape(S // 128, 128, HD).transpose(1, 0, 2).copy()
    sgn = std.copy()
    sgn[:, :, : HD // 2] = -std[:, :, : HD // 2]
    sgn = np.ascontiguousarray(sgn)

    rot = np.zeros((128, 128), dtype=np.float32)
    for i in range(64):
        rot[i, i + 64] = 1.0
        rot[i + 64, i] = -1.0

    p = np.arange(128)[:, None]
    f = np.arange(128)[None, :]
    tril = (p <= f).astype(np.float32)
    m3 = np.concatenate([np.zeros((128, 128), np.float32), tril], axis=1)

    ones = np.ones((128, 128), dtype=np.float32)
    ident = np.eye(128, dtype=np.float32)
    return {
        "cosT": cosT, "sinT": sinT, "ctd": ctd, "sgn": sgn,
        "rot": rot, "tril": tril, "m3": m3, "ones": ones, "ident": ident,
    }


def _build_nc():
    nc = bass.Bass("TRN2", target_bir_lowering=False, debug=False)

    dataT = nc.dram_tensor("dataT", [D, S], BF16, kind="ExternalInput").ap()
    wq = nc.dram_tensor("wq", [D, GQ], BF16, kind="ExternalInput").ap()
    wkv = nc.dram_tensor("wkv", [D, 2 * HD], BF16, kind="ExternalInput").ap()
    wo = nc.dram_tensor("wo", [GQ, D], F32R, kind="ExternalInput").ap()
    cosT_d = nc.dram_tensor("cosT", [128, S], F32, kind="ExternalInput").ap()
    sinT_d = nc.dram_tensor("sinT", [128, S], F32, kind="ExternalInput").ap()
    ctd_d = nc.dram_tensor("ctd", [128, NKO, HD], F32, kind="ExternalInput").ap()
    sgn_d = nc.dram_tensor("sgn", [128, NKO, HD], F32, kind="ExternalInput").ap()
    rot_d = nc.dram_tensor("rot", [128, 128], F32R, kind="ExternalInput").ap()
    tril_d = nc.dram_tensor("tril", [128, 128], F32R, kind="ExternalInput").ap()
    m3_d = nc.dram_tensor("m3", [128, 256], F32R, kind="ExternalInput").ap()
    ones_d = nc.dram_tensor("ones", [128, 128], F32R, kind="ExternalInput").ap()
    ident_d = nc.dram_tensor("ident", [128, 128], F32R, kind="ExternalInput").ap()
    outT = nc.dram_tensor("outT", [D, S], F32, kind="ExternalOutput").ap()

    dataT_r = dataT.rearrange("(ko p) t -> p ko t", p=128)
    wq_r = wq.rearrange("(ko p) m -> p ko m", p=128)
    wkv_r = wkv.rearrange("(ko p) m -> p ko m", p=128)
    wo_r = wo.rearrange("(h p) n -> p h n", p=128)

    from contextlib import ExitStack
    with tile.TileContext(nc) as tc, ExitStack() as stack:
        small_consts = stack.enter_context(tc.tile_pool(name="sconsts", bufs=1))
        rot_sb = small_consts.tile([128, 128], F32R)
        ones_sb = small_consts.tile([128, 128], F32R)
        id_sb = small_consts.tile([128, 128], F32R)
        tril_sb = small_consts.tile([128, 128], F32R)
        m3_sb = small_consts.tile([128, 256], F32R)

        persist = stack.enter_context(tc.tile_pool(name="persist", bufs=1))
        xq4 = persist.tile([128, 4, S], F32R, name="xq4")
        kt4 = persist.tile([128, 4, QC], F32R, name="kt4")
        v_g = [persist.tile([128, 4, HD], F32R, tag=f"vg{g}", name=f"v_g{g}")
               for g in range(4)]

        GRP = 4

        with tc.tile_pool(name="p1consts", bufs=1) as p1c, \
             tc.tile_pool(name="wpool", bufs=1) as wpool, \
             tc.tile_pool(name="datapool", bufs=2) as datapool, \
             tc.tile_pool(name="kvstage", bufs=2) as kvstage, \
             tc.tile_pool(name="qtmp", bufs=2) as qtmp, \
             tc.tile_pool(name="t2pool", bufs=3) as t2pool, \
             tc.tile_pool(name="proj_ps", bufs=3, space="PSUM") as proj_ps, \
             tc.tile_pool(name="kv_ps", bufs=2, space="PSUM") as kv_ps, \
             tc.tile_pool(name="rope_ps", bufs=2, space="PSUM") as rope_ps, \
             tc.tile_pool(name="tp_ps", bufs=1, space="PSUM") as tp_ps:
            cos_sb = p1c.tile([128, S], F32)
            sin_sb = p1c.tile([128, S], F32)
            ctd_sb = p1c.tile([128, NKO, HD], F32)
            sgn_sb = p1c.tile([128, NKO, HD], F32)
            wq_sb = wpool.tile([128, NKO, GQ], BF16)
            wkv_sb = wpool.tile([128, NKO, 2 * HD], BF16)

            dT = {}
            for c in range(2):
                dT[c] = datapool.tile([128, NKO, PC], BF16, tag="dT",
                                      name=f"dT{c}")

            nc.sync.dma_start(wkv_sb[:], wkv_r[:])
            nc.sync.dma_start(dT[0][:, 0:8], dataT_r[:, 0:8, bass.ts(0, PC)])
            nc.gpsimd.dma_start(dT[0][:, 8:16], dataT_r[:, 8:16, bass.ts(0, PC)])
            nc.sync.dma_start(wq_sb[:], wq_r[:])
            nc.gpsimd.dma_start(dT[1][:], dataT_r[:, :, bass.ts(1, PC)])
            nc.sync.dma_start(ctd_sb[:], ctd_d[:])
            nc.sync.dma_start(sgn_sb[:], sgn_d[:])
            nc.sync.dma_start(cos_sb[:], cosT_d[:])
            nc.sync.dma_start(sin_sb[:], sinT_d[:])
            nc.sync.dma_start(rot_sb[:], rot_d[:])
            nc.sync.dma_start(ones_sb[:], ones_d[:])
            nc.sync.dma_start(id_sb[:], ident_d[:])
            nc.sync.dma_start(tril_sb[:], tril_d[:])
            nc.sync.dma_start(m3_sb[:], m3_d[:])

            def quant_group(src_ap, dst_ap):
                amax = qtmp.tile([128, GRP, 1], F32, tag="amax")
                scl = qtmp.tile([128, GRP, 1], F32, tag="scl")
                inv = qtmp.tile([128, GRP, 1], F32, tag="inv")
                xs = qtmp.tile([128, GRP, HD], F32, tag="xs")
                nc.vector.tensor_reduce(amax[:], src_ap, mybir.AxisListType.X,
                                        mybir.AluOpType.max,
                                        apply_absolute_value=True)
                nc.vector.tensor_scalar_max(amax[:], amax[:], 1e-8)
                nc.vector.tensor_scalar_mul(scl[:], amax[:], 1.0 / 127.0)
                nc.vector.reciprocal(inv[:], scl[:])
                sclb = scl[:].to_broadcast((128, GRP, HD))
                invb = inv[:].to_broadcast((128, GRP, HD))
                nc.vector.tensor_tensor(xs[:], src_ap, invb, MULT)
                nc.vector.tensor_scalar_add(xs[:], xs[:], MAGIC)
                nc.vector.tensor_scalar_add(xs[:], xs[:], -MAGIC)
                nc.vector.tensor_tensor(dst_ap, xs[:], sclb, MULT)

            for c in range(NPC):
                csl = bass.ts(c, PC)
                if c + 2 < NPC:
                    cb = c + 2
                    t_ = datapool.tile([128, NKO, PC], BF16, tag="dT",
                                       name=f"dT{cb}")
                    dT[cb] = t_
                    eng = nc.gpsimd if cb % 2 else nc.sync
                    eng.dma_start(t_[:], dataT_r[:, :, bass.ts(cb, PC)])

                kv_td = kvstage.tile([128, GRP, 2 * HD], F32, tag="kvtd",
                                     name=f"kvtd{c}")
                for j in range(GRP):
                    pkv = kv_ps.tile([128, 2 * HD], F32, tag="pkv")
                    for ko in range(NKO):
                        nc.tensor.matmul(pkv[:],
                                         dT[c][:, ko, bass.ds(j * 128, 128)],
                                         wkv_sb[:, ko],
                                         start=(ko == 0), stop=(ko == NKO - 1))
                    nc.scalar.copy(kv_td[:, j, :], pkv[:])

                kr = kvstage.tile([128, GRP, HD], F32, tag="kr", name=f"kr{c}")
                t2k = qtmp.tile([128, GRP, HD], F32, tag="t2k")
                tsl = bass.ts(c, GRP)
                nc.vector.tensor_tensor(kr[:], kv_td[:, :, 0:HD],
                                        ctd_sb[:, tsl], MULT)
                nc.vector.tensor_tensor(t2k[:, :, 0:64],
                                        kv_td[:, :, 64:HD],
                                        sgn_sb[:, tsl, 0:64], MULT)
                nc.vector.tensor_tensor(t2k[:, :, 64:HD],
                                        kv_td[:, :, 0:64],
                                        sgn_sb[:, tsl, 64:HD], MULT)
                nc.vector.tensor_tensor(kr[:], kr[:], t2k[:], ADD)

                kq = kvstage.tile([128, GRP, HD], F32R, tag="kq", name=f"kq{c}")
                quant_group(kr[:], kq[:])
                quant_group(kv_td[:, :, HD:], v_g[c][:])

                for j in range(GRP):
                    pt = tp_ps.tile([128, 128], F32R, tag="tp")
                    nc.tensor.transpose(pt[:], kq[:, j, :], id_sb[:])
                    nc.scalar.copy(kt4[:, c, bass.ts(j, 128)], pt[:])

                for h in range(4):
                    pq = proj_ps.tile([128, QC], F32, tag="pq",
                                      name=f"pq{c}_{h}")
                    for ko in range(NKO):
                        nc.tensor.matmul(pq[:], wq_sb[:, ko, bass.ts(h, 128)],
                                         dT[c][:, ko],
                                         start=(ko == 0), stop=(ko == NKO - 1))
                    nc.scalar.copy(xq4[:, h, csl], pq[:])
                    pr = rope_ps.tile([128, QC], F32, tag="pr")
                    nc.tensor.matmul(pr[:], rot_sb[:], xq4[:, h, csl],
                                     start=True, stop=True)
                    t1 = t2pool.tile([128, QC], F32, tag="t1")
                    t2 = t2pool.tile([128, QC], F32, tag="t2")
                    nc.vector.tensor_tensor(t1[:], xq4[:, h, csl],
                                            cos_sb[:, csl], MULT)
                    nc.vector.tensor_tensor(t2[:], pr[:], sin_sb[:, csl], MULT)
                    nc.vector.tensor_tensor(xq4[:, h, csl], t1[:], t2[:], ADD)

        with tc.tile_pool(name="p2", bufs=1) as p2, \
             tc.tile_pool(name="attn_sb", bufs=8) as attn_sb, \
             tc.tile_pool(name="exp_pool", bufs=12) as exp_pool, \
             tc.tile_pool(name="small", bufs=4) as small, \
             tc.tile_pool(name="outstage", bufs=4) as outstage, \
             tc.tile_pool(name="score_ps", bufs=2, space="PSUM") as score_ps, \
             tc.tile_pool(name="attn_ps", bufs=3, space="PSUM") as attn_ps, \
             tc.tile_pool(name="sum_ps", bufs=2, space="PSUM") as sum_ps:
            out_ps = score_ps
            wo_t = [p2.tile([128, S], F32R, tag=f"wo{h}", name=f"wo{h}")
                    for h in range(4)]
            for h in range(4):
                nc.gpsimd.dma_start(wo_t[h][:], wo_r[:, h])

            def out_proj(c_prev, tiles):
                cpsl = bass.ts(c_prev, QC)
                for dt_ in range(NKO):
                    po = out_ps.tile([128, QC], F32, tag="ps")
                    for h2 in range(4):
                        nc.tensor.matmul(po[:], wo_t[h2][:, bass.ts(dt_, 128)],
                                         tiles[h2][:],
                                         start=(h2 == 0), stop=(h2 == 3))
                    ot = outstage.tile([128, QC], F32, tag="ot")
                    if dt_ % 2 == 0:
                        nc.scalar.copy(ot[:], po[:])
                    else:
                        nc.vector.tensor_copy(ot[:], po[:])
                    nc.sync.dma_start(outT[bass.ts(dt_, 128), cpsl], ot[:])

            prev = None
            for c in range(NQC):
                csl = bass.ts(c, QC)
                nki = 4 * (c + 1)
                attn_tiles = {}
                for h in range(4):
                    pa = attn_ps.tile([128, QC], F32, tag="pa")
                    pss = sum_ps.tile([128, QC], F32, tag="pss")
                    for ki in range(nki):
                        if ki >= 4 * c:
                            j = ki - 4 * c
                            qoff = min(128 * j, 256)
                        else:
                            j, qoff = -1, 0
                        w = QC - qoff
                        ps = score_ps.tile([128, QC], F32, tag="ps")
                        nc.tensor.matmul(ps[:, qoff:],
                                         kt4[:, ki // 4, bass.ts(ki % 4, 128)],
                                         xq4[:, h, bass.ds(c * QC + qoff, w)],
                                         start=True, stop=True)
                        et = exp_pool.tile([128, QC], F32R, tag="et")
                        if j == 3:
                            nc.scalar.activation(et[:, 256:384], ps[:, 256:384],
                                                 EXP, scale=SM_SCALE)
                            nc.scalar.activation(et[:, 384:], ps[:, 384:], EXP,
                                                 scale=SM_SCALE)
                            nc.vector.tensor_tensor(et[:, 256:], et[:, 256:],
                                                    m3_sb[:], MULT)
                        else:
                            nc.scalar.activation(et[:, qoff:], ps[:, qoff:],
                                                 EXP, scale=SM_SCALE)
                            if j >= 0:
                                lo = 128 * j
                                nc.vector.tensor_tensor(
                                    et[:, lo:lo + 128], et[:, lo:lo + 128],
                                    tril_sb[:], MULT)
                        nc.tensor.matmul(pss[:, qoff:], ones_sb[:],
                                         et[:, qoff:],
                                         start=(ki == 0), stop=(ki == nki - 1))
                        nc.tensor.matmul(pa[:, qoff:], v_g[ki // 4][:, ki % 4],
                                         et[:, qoff:],
                                         start=(ki == 0), stop=(ki == nki - 1))
                    rc = small.tile([128, QC], F32, tag="rc")
                    nc.vector.reciprocal(rc[:], pss[:])
                    at = attn_sb.tile([128, QC], F32R, tag="attnT")
                    attn_tiles[h] = at
                    nc.vector.tensor_tensor(at[:], pa[:], rc[:], MULT)
                    if h == 1 and prev is not None:
                        out_proj(prev[0], prev[1])
                prev = (c, attn_tiles)
            out_proj(prev[0], prev[1])

    _split_multi_waits(nc)
    return nc


def _get_state():
    if "nc" not in _CACHE:
        _CACHE["nc"] = _build_nc()
        _CACHE["consts"] = _host_consts()
    return _CACHE["nc"], _CACHE["consts"]


def kernel(data=None, mask=None, wq=None, wk=None, wv=None, wo=None, **extra):
    global LAST_RESULTS
    import ml_dtypes
    bf16 = ml_dtypes.bfloat16
    nc, consts = _get_state()

    data = np.asarray(data, dtype=np.float32)
    wq = np.asarray(wq, dtype=np.float32)
    wk = np.asarray(wk, dtype=np.float32)
    wv = np.asarray(wv, dtype=np.float32)
    wo = np.asarray(wo, dtype=np.float32)

    in_maps = []
    dTs = [np.ascontiguousarray(data[b].T).astype(bf16) for b in range(B)]
    for b in range(B):
        for g in range(NKV):
            in_maps.append({
                "dataT": dTs[b],
                "wq": wq[:, g * GQ:(g + 1) * GQ].astype(bf16),
                "wkv": np.ascontiguousarray(np.concatenate(
                    [wk[:, g * HD:(g + 1) * HD],
                     wv[:, g * HD:(g + 1) * HD]], axis=1)).astype(bf16),
                "wo": np.ascontiguousarray(wo[g * GQ:(g + 1) * GQ, :]),
                "cosT": consts["cosT"],
                "sinT": consts["sinT"],
                "ctd": consts["ctd"],
                "sgn": consts["sgn"],
                "rot": consts["rot"],
                "tril": consts["tril"],
                "m3": consts["m3"],
                "ones": consts["ones"],
                "ident": consts["ident"],
            })

    res = run_bass_kernel_spmd(nc, in_maps, core_ids=list(range(8)))
    LAST_RESULTS = res

    out = np.empty((B, S, D), dtype=np.float32)
    for b in range(B):
        acc = res.results[b * NKV]["outT"].astype(np.float32).copy()
        for g in range(1, NKV):
            acc += res.results[b * NKV + g]["outT"]
        out[b] = acc.T
    return out


# revision 6
# speedup vs baseline: 1.1761x; 1.0434x over previous
import numpy as np

import bass_rust
import concourse.bass as bass
import concourse.tile as tile
import concourse.mybir as mybir
from concourse.bass_utils import run_bass_kernel_spmd

B, S, D = 2, 2048, 2048
NH, NKV, HD = 16, 4, 128
GQ = 512
NKO = D // 128
PC = 512
NPC = S // PC
QC = 512
NQC = S // QC
MAGIC = float(np.float32(12582912.0))
SM_SCALE = 1.0 / float(np.sqrt(HD))

F32 = mybir.dt.float32
F32R = mybir.dt.float32r
BF16 = mybir.dt.bfloat16
MULT = mybir.AluOpType.mult
ADD = mybir.AluOpType.add
EXP = mybir.ActivationFunctionType.Exp

_CACHE = {}

LAST_RESULTS = None


def _split_multi_waits(nc):
    for f in nc.m.functions:
        for bb in f.blocks:
            new = []
            for inst in bb.instructions:
                si = inst.sync_info
                if si is None:
                    new.append(inst)
                    continue
                waits = list(si.on_wait)
                if len(waits) > 1:
                    for k, w in enumerate(waits[:-1]):
                        nop = mybir.InstNoOp(name=f"{inst.name}-w{k}", ins=[], outs=[])
                        nop.engine = inst.engine
                        nop.sync_info = bass_rust.SyncInfo(on_wait=[w], on_update=[])
                        new.append(nop)
                    inst.sync_info = bass_rust.SyncInfo(
                        on_wait=[waits[-1]], on_update=list(si.on_update)
                    )
                new.append(inst)
            bb.instructions = new


def _host_consts():
    theta = 10000.0
    angles = 1.0 / theta ** (np.arange(0, HD, 2, dtype=np.float32) / HD)
    emb = np.outer(np.arange(S, dtype=np.float32), angles)
    emb = np.concatenate([emb, emb], axis=-1)
    cos = np.cos(emb).astype(np.float32)
    sin = np.sin(emb).astype(np.float32)
    cosT = np.ascontiguousarray(cos.T)
    sinT = np.ascontiguousarray(sin.T)

    ctd = np.ascontiguousarray(cos.reshape(S // 128, 128, HD).transpose(1, 0, 2))
    std = sin.reshape(S // 128, 128, HD).transpose(1, 0, 2).copy()
    sgn = std.copy()
    sgn[:, :, : HD // 2] = -std[:, :, : HD // 2]
    sgn = np.ascontiguousarray(sgn)

    rot = np.zeros((128, 128), dtype=np.float32)
    for i in range(64):
        rot[i, i + 64] = 1.0
        rot[i + 64, i] = -1.0

    p = np.arange(128)[:, None]
    f = np.arange(128)[None, :]
    tril = (p <= f).astype(np.float32)
    m3 = np.concatenate([np.zeros((128, 128), np.float32), tril], axis=1)

    ones = np.ones((128, 128), dtype=np.float32)
    ident = np.eye(128, dtype=np.float32)
    return {
        "cosT": cosT, "sinT": sinT, "ctd": ctd, "sgn": sgn,
        "rot": rot, "tril": tril, "m3": m3, "ones": ones, "ident": ident,
    }


def _build_nc():
    nc = bass.Bass("TRN2", target_bir_lowering=False, debug=False)

    dataT = nc.dram_tensor("dataT", [D, S], BF16, kind="ExternalInput").ap()
    wq = nc.dram_tensor("wq", [D, GQ], BF16, kind="ExternalInput").ap()
    wkv = nc.dram_tensor("wkv", [D, 2 * HD], BF16, kind="ExternalInput").ap()
    wo = nc.dram_tensor("wo", [GQ, D], F32R, kind="ExternalInput").ap()
    cosT_d = nc.dram_tensor("cosT", [128, S], F32, kind="ExternalInput").ap()
    sinT_d = nc.dram_tensor("sinT", [128, S], F32, kind="ExternalInput").ap()
    ctd_d = nc.dram_tensor("ctd", [128, NKO, HD], F32, kind="ExternalInput").ap()
    sgn_d = nc.dram_tensor("sgn", [128, NKO, HD], F32, kind="ExternalInput").ap()
    rot_d = nc.dram_tensor("rot", [128, 128], F32R, kind="ExternalInput").ap()
    tril_d = nc.dram_tensor("tril", [128, 128], F32R, kind="ExternalInput").ap()
    m3_d = nc.dram_tensor("m3", [128, 256], F32R, kind="ExternalInput").ap()
    ones_d = nc.dram_tensor("ones", [128, 128], F32R, kind="ExternalInput").ap()
    ident_d = nc.dram_tensor("ident", [128, 128], F32R, kind="ExternalInput").ap()
    outT = nc.dram_tensor("outT", [D, S], F32, kind="ExternalOutput").ap()

    dataT_r = dataT.rearrange("(ko p) t -> p ko t", p=128)
    wq_r = wq.rearrange("(ko p) m -> p ko m", p=128)
    wkv_r = wkv.rearrange("(ko p) m -> p ko m", p=128)
    wo_r = wo.rearrange("(h p) n -> p h n", p=128)

    from contextlib import ExitStack
    with tile.TileContext(nc) as tc, ExitStack() as stack:
        small_consts = stack.enter_context(tc.tile_pool(name="sconsts", bufs=1))
        rot_sb = small_consts.tile([128, 128], F32R)
        ones_sb = small_consts.tile([128, 128], F32R)
        id_sb = small_consts.tile([128, 128], F32R)
        tril_sb = small_consts.tile([128, 128], F32R)
        m3_sb = small_consts.tile([128, 256], F32R)

        persist = stack.enter_context(tc.tile_pool(name="persist", bufs=1))
        xq4 = persist.tile([128, 4, S], F32R, name="xq4")
        kt4 = persist.tile([128, 4, QC], F32R, name="kt4")
        v_g = [persist.tile([128, 4, HD], F32R, tag=f"vg{g}", name=f"v_g{g}")
               for g in range(4)]

        GRP = 4

        with tc.tile_pool(name="p1consts", bufs=1) as p1c, \
             tc.tile_pool(name="wpool", bufs=1) as wpool, \
             tc.tile_pool(name="datapool", bufs=2) as datapool, \
             tc.tile_pool(name="kvstage", bufs=2) as kvstage, \
             tc.tile_pool(name="qtmp", bufs=2) as qtmp, \
             tc.tile_pool(name="t2pool", bufs=3) as t2pool, \
             tc.tile_pool(name="proj_ps", bufs=3, space="PSUM") as proj_ps, \
             tc.tile_pool(name="kv_ps", bufs=2, space="PSUM") as kv_ps, \
             tc.tile_pool(name="rope_ps", bufs=2, space="PSUM") as rope_ps, \
             tc.tile_pool(name="tp_ps", bufs=1, space="PSUM") as tp_ps:
            cos_sb = p1c.tile([128, S], F32)
            sin_sb = p1c.tile([128, S], F32)
            ctd_sb = p1c.tile([128, NKO, HD], F32)
            sgn_sb = p1c.tile([128, NKO, HD], F32)
            wq_sb = wpool.tile([128, NKO, GQ], BF16)
            wkv_sb = wpool.tile([128, NKO, 2 * HD], BF16)

            dT = {}
            for c in range(2):
                dT[c] = datapool.tile([128, NKO, PC], BF16, tag="dT",
                                      name=f"dT{c}")

            nc.sync.dma_start(wkv_sb[:], wkv_r[:])
            nc.sync.dma_start(dT[0][:, 0:8], dataT_r[:, 0:8, bass.ts(0, PC)])
            nc.gpsimd.dma_start(dT[0][:, 8:16], dataT_r[:, 8:16, bass.ts(0, PC)])
            nc.sync.dma_start(wq_sb[:], wq_r[:])
            nc.gpsimd.dma_start(dT[1][:], dataT_r[:, :, bass.ts(1, PC)])
            nc.sync.dma_start(ctd_sb[:], ctd_d[:])
            nc.sync.dma_start(sgn_sb[:], sgn_d[:])
            nc.sync.dma_start(cos_sb[:], cosT_d[:])
            nc.sync.dma_start(sin_sb[:], sinT_d[:])
            nc.sync.dma_start(rot_sb[:], rot_d[:])
            nc.sync.dma_start(ones_sb[:], ones_d[:])
            nc.sync.dma_start(id_sb[:], ident_d[:])
            nc.sync.dma_start(tril_sb[:], tril_d[:])
            nc.sync.dma_start(m3_sb[:], m3_d[:])

            def quant_group(src_ap, dst_ap):
                amax = qtmp.tile([128, GRP, 1], F32, tag="amax")
                scl = qtmp.tile([128, GRP, 1], F32, tag="scl")
                inv = qtmp.tile([128, GRP, 1], F32, tag="inv")
                xs = qtmp.tile([128, GRP, HD], F32, tag="xs")
                nc.vector.tensor_reduce(amax[:], src_ap, mybir.AxisListType.X,
                                        mybir.AluOpType.max,
                                        apply_absolute_value=True)
                nc.vector.tensor_scalar_max(amax[:], amax[:], 1e-8)
                nc.vector.tensor_scalar_mul(scl[:], amax[:], 1.0 / 127.0)
                nc.vector.reciprocal(inv[:], scl[:])
                sclb = scl[:].to_broadcast((128, GRP, HD))
                invb = inv[:].to_broadcast((128, GRP, HD))
                nc.vector.tensor_tensor(xs[:], src_ap, invb, MULT)
                nc.vector.tensor_scalar_add(xs[:], xs[:], MAGIC)
                nc.vector.tensor_scalar_add(xs[:], xs[:], -MAGIC)
                nc.vector.tensor_tensor(dst_ap, xs[:], sclb, MULT)

            for c in range(NPC):
                csl = bass.ts(c, PC)
                if c + 2 < NPC:
                    cb = c + 2
                    t_ = datapool.tile([128, NKO, PC], BF16, tag="dT",
                                       name=f"dT{cb}")
                    dT[cb] = t_
                    eng = nc.gpsimd if cb % 2 else nc.sync
                    eng.dma_start(t_[:, 0:8], dataT_r[:, 0:8, bass.ts(cb, PC)])
                    eng.dma_start(t_[:, 8:16], dataT_r[:, 8:16, bass.ts(cb, PC)])

                kv_td = kvstage.tile([128, GRP, 2 * HD], F32, tag="kvtd",
                                     name=f"kvtd{c}")
                for j in range(GRP):
                    pkv = kv_ps.tile([128, 2 * HD], F32, tag="pkv")
                    for ko in range(NKO):
                        nc.tensor.matmul(pkv[:],
                                         dT[c][:, ko, bass.ds(j * 128, 128)],
                                         wkv_sb[:, ko],
                                         start=(ko == 0), stop=(ko == NKO - 1))
                    nc.scalar.copy(kv_td[:, j, :], pkv[:])

                kr = kvstage.tile([128, GRP, HD], F32, tag="kr", name=f"kr{c}")
                t2k = qtmp.tile([128, GRP, HD], F32, tag="t2k")
                tsl = bass.ts(c, GRP)
                nc.vector.tensor_tensor(kr[:], kv_td[:, :, 0:HD],
                                        ctd_sb[:, tsl], MULT)
                nc.vector.tensor_tensor(t2k[:, :, 0:64],
                                        kv_td[:, :, 64:HD],
                                        sgn_sb[:, tsl, 0:64], MULT)
                nc.vector.tensor_tensor(t2k[:, :, 64:HD],
                                        kv_td[:, :, 0:64],
                                        sgn_sb[:, tsl, 64:HD], MULT)
                nc.vector.tensor_tensor(kr[:], kr[:], t2k[:], ADD)

                kq = kvstage.tile([128, GRP, HD], F32R, tag="kq", name=f"kq{c}")
                quant_group(kr[:], kq[:])
                quant_group(kv_td[:, :, HD:], v_g[c][:])

                def emit_qproj(h):
                    pq = proj_ps.tile([128, QC], F32, tag="pq",
                                      name=f"pq{c}_{h}")
                    for ko in range(NKO):
                        nc.tensor.matmul(pq[:], wq_sb[:, ko, bass.ts(h, 128)],
                                         dT[c][:, ko],
                                         start=(ko == 0), stop=(ko == NKO - 1))
                    nc.scalar.copy(xq4[:, h, csl], pq[:])

                emit_qproj(0)
                emit_qproj(1)
                for j in range(GRP):
                    pt = tp_ps.tile([128, 128], F32R, tag="tp")
                    nc.tensor.transpose(pt[:], kq[:, j, :], id_sb[:])
                    nc.scalar.copy(kt4[:, c, bass.ts(j, 128)], pt[:])
                emit_qproj(2)
                emit_qproj(3)
                for h in range(4):
                    pr = rope_ps.tile([128, QC], F32, tag="pr")
                    nc.tensor.matmul(pr[:], rot_sb[:], xq4[:, h, csl],
                                     start=True, stop=True)
                    t1 = t2pool.tile([128, QC], F32, tag="t1")
                    t2 = t2pool.tile([128, QC], F32, tag="t2")
                    nc.vector.tensor_tensor(t1[:], xq4[:, h, csl],
                                            cos_sb[:, csl], MULT)
                    nc.vector.tensor_tensor(t2[:], pr[:], sin_sb[:, csl], MULT)
                    nc.vector.tensor_tensor(xq4[:, h, csl], t1[:], t2[:], ADD)

        with tc.tile_pool(name="p2", bufs=1) as p2, \
             tc.tile_pool(name="attn_sb", bufs=8) as attn_sb, \
             tc.tile_pool(name="exp_pool", bufs=12) as exp_pool, \
             tc.tile_pool(name="small", bufs=4) as small, \
             tc.tile_pool(name="outstage", bufs=4) as outstage, \
             tc.tile_pool(name="score_ps", bufs=4, space="PSUM") as score_ps, \
             tc.tile_pool(name="attn_ps", bufs=2, space="PSUM") as attn_ps, \
             tc.tile_pool(name="sum_ps", bufs=2, space="PSUM") as sum_ps:
            out_ps = score_ps
            wo_t = [p2.tile([128, S], F32R, tag=f"wo{h}", name=f"wo{h}")
                    for h in range(4)]
            for h in range(4):
                nc.sync.dma_start(wo_t[h][:], wo_r[:, h])

            def out_proj(c_prev, tiles):
                cpsl = bass.ts(c_prev, QC)
                for dt_ in range(NKO):
                    po = out_ps.tile([128, QC], F32, tag="ps")
                    for h2 in range(4):
                        nc.tensor.matmul(po[:], wo_t[h2][:, bass.ts(dt_, 128)],
                                         tiles[h2][:],
                                         start=(h2 == 0), stop=(h2 == 3))
                    ot = outstage.tile([128, QC], F32, tag="ot")
                    if dt_ % 2 == 0:
                        nc.scalar.copy(ot[:], po[:])
                    else:
                        nc.vector.tensor_copy(ot[:], po[:])
                    nc.sync.dma_start(outT[bass.ts(dt_, 128), cpsl], ot[:])

            prev = None

            def emit_pair(c, hA, hB, attn_tiles):
                nki = 4 * (c + 1)
                streams = (hA, hB)
                pa = [attn_ps.tile([128, QC], F32, tag="pa",
                                   name=f"pa{c}_{h}") for h in streams]
                pss = [sum_ps.tile([128, QC], F32, tag="pss",
                                   name=f"pss{c}_{h}") for h in streams]

                def emit_acc(st, ki, et, qoff):
                    nc.tensor.matmul(pss[st][:, qoff:], ones_sb[:],
                                     et[:, qoff:],
                                     start=(ki == 0), stop=(ki == nki - 1))
                    nc.tensor.matmul(pa[st][:, qoff:],
                                     v_g[ki // 4][:, ki % 4], et[:, qoff:],
                                     start=(ki == 0), stop=(ki == nki - 1))

                pending = []
                for ki in range(nki):
                    if ki >= 4 * c:
                        j = ki - 4 * c
                        qoff = min(128 * j, 256)
                    else:
                        j, qoff = -1, 0
                    w = QC - qoff
                    for st in range(2):
                        h = streams[st]
                        ps = score_ps.tile([128, QC], F32, tag="ps")
                        nc.tensor.matmul(ps[:, qoff:],
                                         kt4[:, ki // 4, bass.ts(ki % 4, 128)],
                                         xq4[:, h, bass.ds(c * QC + qoff, w)],
                                         start=True, stop=True)
                        et = exp_pool.tile([128, QC], F32R, tag="et")
                        nc.scalar.activation(et[:, qoff:], ps[:, qoff:], EXP,
                                             scale=SM_SCALE)
                        if j == 3:
                            nc.gpsimd.tensor_tensor(et[:, 256:], et[:, 256:],
                                                    m3_sb[:], MULT)
                        elif j >= 0:
                            lo = 128 * j
                            nc.gpsimd.tensor_tensor(
                                et[:, lo:lo + 128], et[:, lo:lo + 128],
                                tril_sb[:], MULT)
                        pending.append((st, ki, et, qoff))
                    if ki >= 1:
                        emit_acc(*pending.pop(0))
                        emit_acc(*pending.pop(0))
                for item in pending:
                    emit_acc(*item)
                for st in range(2):
                    rc = small.tile([128, QC], F32, tag="rc")
                    nc.vector.reciprocal(rc[:], pss[st][:])
                    at = attn_sb.tile([128, QC], F32R, tag="attnT")
                    attn_tiles[streams[st]] = at
                    nc.vector.tensor_tensor(at[:], pa[st][:], rc[:], MULT)

            for c in range(NQC):
                attn_tiles = {}
                emit_pair(c, 0, 1, attn_tiles)
                if prev is not None:
                    out_proj(prev[0], prev[1])
                emit_pair(c, 2, 3, attn_tiles)
                prev = (c, attn_tiles)
            out_proj(prev[0], prev[1])

    _split_multi_waits(nc)
    return nc


def _get_state():
    if "nc" not in _CACHE:
        _CACHE["nc"] = _build_nc()
        _CACHE["consts"] = _host_consts()
    return _CACHE["nc"], _CACHE["consts"]


def kernel(data=None, mask=None, wq=None, wk=None, wv=None, wo=None, **extra):
    global LAST_RESULTS
    import ml_dtypes
    bf16 = ml_dtypes.bfloat16
    nc, consts = _get_state()

    data = np.asarray(data, dtype=np.float32)
    wq = np.asarray(wq, dtype=np.float32)
    wk = np.asarray(wk, dtype=np.float32)
    wv = np.asarray(wv, dtype=np.float32)
    wo = np.asarray(wo, dtype=np.float32)

    in_maps = []
    dTs = [np.ascontiguousarray(data[b].T).astype(bf16) for b in range(B)]
    for b in range(B):
        for g in range(NKV):
            in_maps.append({
                "dataT": dTs[b],
                "wq": wq[:, g * GQ:(g + 1) * GQ].astype(bf16),
                "wkv": np.ascontiguousarray(np.concatenate(
                    [wk[:, g * HD:(g + 1) * HD],
                     wv[:, g * HD:(g + 1) * HD]], axis=1)).astype(bf16),
                "wo": np.ascontiguousarray(wo[g * GQ:(g + 1) * GQ, :]),
                "cosT": consts["cosT"],
                "sinT": consts["sinT"],
                "ctd": consts["ctd"],
                "sgn": consts["sgn"],
                "rot": consts["rot"],
                "tril": consts["tril"],
                "m3": consts["m3"],
                "ones": consts["ones"],
                "ident": consts["ident"],
            })

    res = run_bass_kernel_spmd(nc, in_maps, core_ids=list(range(8)))
    LAST_RESULTS = res

    out = np.empty((B, S, D), dtype=np.float32)
    for b in range(B):
        acc = res.results[b * NKV]["outT"].astype(np.float32).copy()
        for g in range(1, NKV):
            acc += res.results[b * NKV + g]["outT"]
        out[b] = acc.T
    return out
